# revision 33
# baseline (speedup 1.0000x reference)
"""JPEG encoder Bass kernel for TRN2 — self-contained, 8-core data-parallel.

kernel(img, D, Q) -> (flatten, no_quan_flatten), matching the reference:
    per 8x8 block: dct = D @ (X - 128) @ D.T ; quant = round(dct / Q);
    both zigzag-gathered + channel-concatenated to (256, 512, 192).

Device computes ONLY the unquantized DCT as int16 (= 8 * dct(X), which
always fits: |8*dct| <= 16320); the host derives nq = int16/8 with a DC
fix (dct(X) = dct(X-128) + 1024*delta_00) and quant = round(nq/Q). This
halves output DMA vs shipping a separate int8 quant stream and deletes
the whole on-device quant path.

Per 8x8-block-row-pair strip (128 partitions = (brp, b), free = (c,i,w)):
  1 input DMA (2KB runs) -> DVE regroup to z-grouped layout (3 strided
  copies, 4x DVE mode) -> 24 PE transposes (contiguous 128-chunks) ->
  px PSUM f16 -> copies to SBUF (DVE/Pool 2x) -> 6 single-shot fp16
  matmuls per quarter against one block-diag(8*kron(D,D)[zz].T) weight
  (c0|c1 full 128-col + merged c2 pair via block-diag) -> f32 PSUM ->
  f32->int16 cast copies (Act/DVE) -> 2 output DMAs (3KB runs).

Schedule: quarters pipelined with transposes emitted 2 quarters ahead on
PE; input DMAs prefetch on the SP queue, output DMAs issue from the
scalar queue; engine work balanced DVE/Act/Pool so the ~35us of DMA
traffic (6.3MB in + 6.3MB out per core) is the only roofline.
"""

import numpy as np
import concourse.mybir as mybir
import concourse.tile as tile
from concourse import bacc
from concourse.bass_utils import run_bass_kernel_spmd

F32 = mybir.dt.float32
F16 = mybir.dt.float16
I16 = mybir.dt.int16
P = 8
B, C, H, W = 512, 3, 128, 128
NCORES = 8
BSH = B // NCORES          # 64 batches per core
N = (H // P) * (W // P)    # 256 blocks per plane
CZ = C * P * P             # 192
NBR = H // P               # 16 block rows
NSTRIP = NBR // 2          # 8 strips of 2 block rows

Copy = mybir.ActivationFunctionType.Copy


def _zigzag_flat_idx(n=P):
    order = []
    for s in range(2 * n - 1):
        cells = [(r, s - r) for r in range(max(0, s - n + 1), min(s, n - 1) + 1)]
        if s % 2 == 0:
            cells.reverse()
        order.extend(cells)
    return np.array([r * n + c for r, c in order], dtype=np.int32)


def _build_consts(D: np.ndarray):
    ZZ = _zigzag_flat_idx()
    D64 = D.astype(np.float64)
    KD = np.kron(D64, D64)[ZZ, :]          # (64 zz, 64 pix)
    Mt = (16.0 * KD.T).astype(np.float16)  # (pix, zz), x16 output scale
    Wm = np.zeros((128, 128), dtype=np.float16)
    Wm[0:64, 0:64] = Mt
    Wm[64:128, 64:128] = Mt
    bdid = np.eye(128, dtype=np.float16)
    return Wm, bdid


def _build_nc():
    nc = bacc.Bacc("TRN2", target_bir_lowering=False, debug=False)

    img = nc.dram_tensor("img", [BSH, C, H, W], F16, kind="ExternalInput")
    W_d = nc.dram_tensor("Wm", [128, 128], F16, kind="ExternalInput")
    bdid_d = nc.dram_tensor("bdid", [128, 128], F16, kind="ExternalInput")
    # device layout: [strip, brp, b, (Q, c01|c2 packed)]; host reassembles
    out_d = nc.dram_tensor("out", [NSTRIP, 2, BSH, 3072], I16,
                           kind="ExternalOutput")

    # partition = (brp, b), free = (c, (i, w)) with 2KB contiguous runs
    imgv = img[:].rearrange("b c (bp brp i) w -> bp brp b c (i w)",
                            brp=2, i=P)

    with tile.TileContext(nc) as tc:
        with (
            tc.tile_pool(name="const", bufs=1) as constp,
            tc.tile_pool(name="sload", bufs=4) as sload,
            tc.tile_pool(name="greg", bufs=3) as greg,
            tc.tile_pool(name="x", bufs=6) as xp,
            tc.tile_pool(name="sb", bufs=3) as sbp,
            tc.tile_pool(name="px", bufs=2, space="PSUM") as pxp,
            tc.tile_pool(name="po", bufs=2, space="PSUM") as pop,
        ):
            Wm = constp.tile([128, 128], F16)
            bdid = constp.tile([128, 128], F16)

            S_t = [None] * NSTRIP
            G_t = [None] * NSTRIP
            x_t = [None] * (NSTRIP * 4)
            px_t = [None] * (NSTRIP * 4)
            po_t = [None] * (NSTRIP * 2)
            sb_t = [None] * NSTRIP

            def load_s(bp, split=False):
                S = sload.tile([128, 3072], F16, tag="s", name=f"s{bp}")
                if split:
                    # per-c-plane DMAs on three queues so the regroup
                    # copies can start as early as possible
                    Sv3 = S[:].rearrange("p (c iw) -> p c iw", c=3, iw=1024)
                    nc.sync.dma_start(out=Sv3[:, 0], in_=imgv[bp, :, :, 0])
                    nc.gpsimd.dma_start(out=Sv3[:, 1], in_=imgv[bp, :, :, 1])
                    nc.sync.dma_start(out=Sv3[:, 2], in_=imgv[bp, :, :, 2])
                else:
                    nc.sync.dma_start(out=S[:], in_=imgv[bp])
                S_t[bp] = S

            def regroup(bp, part=None):
                # S free (c,i,q,k,j) -> G (q, z, i, j); z = 2k+c | 4+k
                # part 0/1: c01 k-halves; part 2: c2 (emitted first).
                if part in (2, None):
                    G_t[bp] = greg.tile([128, 3072], F16, tag="g",
                                        name=f"g{bp}")
                G = G_t[bp]
                gv = G[:].rearrange("p (q z i j) -> p z q i j",
                                    q=8, z=6, i=P, j=P)
                sv = S_t[bp][:].rearrange("p (c i q k j) -> p k c q i j",
                                          c=3, i=P, q=8, k=2, j=P)
                parts = range(3) if part is None else [part]
                for pt in parts:
                    if pt < 2:
                        nc.vector.tensor_copy(gv[:, 2 * pt:2 * pt + 2],
                                              sv[:, pt, 0:2])
                    elif part is None:
                        nc.vector.tensor_copy(gv[:, 4:6], sv[:, :, 2])
                    else:
                        # c2 regroup off the DVE critical chain: gpsimd is
                        # idle mid-strip and strips 6-7 need no regroup
                        nc.gpsimd.tensor_copy(gv[:, 4:6], sv[:, :, 2])

            def transposes(u):
                bp, Q = u // 4, u % 4
                G = G_t[bp]
                px = pxp.tile([128, 768], F16, tag="px", name=f"px{u}")
                for t in range(2):
                    q = Q * 2 + t
                    for k in range(2):
                        nc.tensor.matmul(
                            px[:, (t * 2 + k) * 128:(t * 2 + k + 1) * 128],
                            G[:, q * 384 + k * 128:q * 384 + (k + 1) * 128],
                            bdid[:], is_transpose=True)
                    nc.tensor.matmul(
                        px[:, 512 + t * 128:512 + (t + 1) * 128],
                        G[:, q * 384 + 256:q * 384 + 384],
                        bdid[:], is_transpose=True)
                px_t[u] = px

            def xcopy(u):
                bp, Q = u // 4, u % 4
                x = xp.tile([128, 768], F16, tag="x", name=f"x{u}")
                nc.vector.tensor_copy(x[:], px_t[u][:])
                x_t[u] = x

            def matmuls(u):
                # half-strip po tile: [c01 8x128 | c2 4x128] per half
                x = x_t[u]
                h, lq = u // 2, u % 2
                if lq == 0:
                    po_t[h] = pop.tile([128, 1536], F32, tag="po",
                                       name=f"po{h}")
                po = po_t[h]
                for t in range(4):
                    bwl = lq * 4 + t
                    nc.tensor.matmul(po[:, bwl * 128:(bwl + 1) * 128],
                                     x[:, t * 128:(t + 1) * 128], Wm[:],
                                     start=True, stop=True)
                for t in range(2):
                    p = lq * 2 + t
                    nc.tensor.matmul(
                        po[:, 1024 + p * 128:1024 + (p + 1) * 128],
                        x[:, 512 + t * 128:512 + (t + 1) * 128],
                        Wm[:], start=True, stop=True)

            def outcopy(h, last=False):
                bp, hh = h // 2, h % 2
                if hh == 0:
                    sb_t[bp] = sbp.tile([128, 3072], I16, tag="sb",
                                        name=f"sb{bp}")
                dst = sb_t[bp][:, hh * 1536:(hh + 1) * 1536]
                if last:
                    # final half: split Act/DVE to shorten the tail chain
                    nc.scalar.activation(dst.rearrange(
                        "p (a f) -> p a f", a=2)[:, 0], po_t[h][:, 0:768],
                        Copy)
                    nc.vector.tensor_copy(dst.rearrange(
                        "p (a f) -> p a f", a=2)[:, 1], po_t[h][:, 768:1536])
                else:
                    nc.scalar.activation(dst, po_t[h][:], Copy)

            def out_dma(bp, half, last=False):
                if last:
                    # final half: two parallel-queue DMAs to shorten tail
                    nc.gpsimd.dma_start(out=out_d[bp, :, :, 1536:2304],
                                        in_=sb_t[bp][:, 1536:2304])
                    nc.sync.dma_start(out=out_d[bp, :, :, 2304:3072],
                                      in_=sb_t[bp][:, 2304:3072])
                    return
                fsl = slice(half * 1536, (half + 1) * 1536)
                if half == 0:
                    nc.gpsimd.dma_start(out=out_d[bp, :, :, fsl],
                                        in_=sb_t[bp][:, fsl])
                else:
                    # h1 on SP: keeps the gpsimd queue clear so the next
                    # strip's c2 regroup isn't stuck behind a desc-gen wait
                    nc.sync.dma_start(out=out_d[bp, :, :, fsl],
                                      in_=sb_t[bp][:, fsl])

            # ---- software-pipelined emission over 32 quarters ----
            # consts first: bdid gates the very first transpose
            nc.scalar.dma_start(out=bdid[:], in_=bdid_d[:])
            nc.scalar.dma_start(out=Wm[:], in_=W_d[:])
            load_s(0, split=True)
            regroup(0)
            load_s(1)
            transposes(0)
            transposes(1)
            NU = NSTRIP * 4
            for u in range(NU):
                bp, Q = u // 4, u % 4
                if Q == 2 and bp + 2 < NSTRIP:
                    load_s(bp + 2)
                xcopy(u)
                if Q < 3 and bp + 1 < NSTRIP:
                    # Q0 -> c2 on gpsimd (ahead of this strip's out-DMAs in
                    # the gpsimd queue), Q1/Q2 -> c01 halves on DVE
                    regroup(bp + 1, part=(2, 0, 1)[Q])
                matmuls(u)
                if u + 2 < NU:
                    transposes(u + 2)
                if Q in (1, 3):
                    h = u // 2
                    last = u == NU - 1
                    outcopy(h, last=last)
                    out_dma(bp, h % 2, last=last)

    nc.compile()
    return nc


_NC_CACHE = None


def _get_nc():
    global _NC_CACHE
    if _NC_CACHE is None:
        _NC_CACHE = _build_nc()
    return _NC_CACHE


def _build_perm():
    # device free offset for (bw, czz): half h = bw//8, local bwl = bw%8;
    # c01 at h*1536 + bwl*128 + c*64 + zz; c2 at h*1536 + 1024 + bwl*64 + zz
    perm = np.zeros(NBR * CZ, dtype=np.int64)
    for bw in range(16):
        h, bwl = bw // 8, bw % 8
        for c in range(3):
            for zz in range(64):
                col = bw * CZ + c * 64 + zz
                if c < 2:
                    off = h * 1536 + bwl * 128 + c * 64 + zz
                else:
                    off = h * 1536 + 1024 + bwl * 64 + zz
                perm[col] = off
    return perm


_PERM = _build_perm()


def kernel(img, D, Q):
    img = np.asarray(img, dtype=np.float32)
    D = np.asarray(D, dtype=np.float32)
    Q = np.asarray(Q, dtype=np.float32)
    Wm, bdid = _build_consts(D)
    ZZ = _zigzag_flat_idx()
    q_zz = np.tile(Q.flatten()[ZZ], C).astype(np.float32)     # (192,)

    # subtract 128 on host: halves fp16 input/weight noise and keeps
    # |16*dct| <= 16384 in int16 (dct of X-128 matches the reference)
    img16 = np.ascontiguousarray(img - np.float32(128.0)).astype(np.float16)
    nc = _get_nc()
    in_maps = [
        {"img": img16[kk * BSH:(kk + 1) * BSH], "Wm": Wm, "bdid": bdid}
        for kk in range(NCORES)
    ]
    res = run_bass_kernel_spmd(nc, in_maps, core_ids=list(range(NCORES)))

    parts = []
    for r in res.results:
        dev = np.asarray(r["out"])                 # (8, 2, 64, 3072) i16
        f = dev[..., _PERM].astype(np.float32)     # (8, 2, 64, 16*192)
        f = f.reshape(NSTRIP, 2, BSH, NBR, CZ)
        f = f.transpose(0, 1, 3, 2, 4).reshape(N, BSH, CZ)
        parts.append(f)
    nq = np.concatenate(parts, axis=1) * np.float32(0.0625)   # (256, 512, 192)
    flatten = np.round(nq / q_zz)
    return (flatten, nq)


# revision 34
# speedup vs baseline: 1.0000x; 1.0000x over previous
"""JPEG encoder Bass kernel for TRN2 — self-contained, 8-core data-parallel.

kernel(img, D, Q) -> (flatten, no_quan_flatten), matching the reference:
    per 8x8 block: dct = D @ (X - 128) @ D.T ; quant = round(dct / Q);
    both zigzag-gathered + channel-concatenated to (256, 512, 192).

Device computes ONLY the unquantized DCT as int16 (= 8 * dct(X), which
always fits: |8*dct| <= 16320); the host derives nq = int16/8 with a DC
fix (dct(X) = dct(X-128) + 1024*delta_00) and quant = round(nq/Q). This
halves output DMA vs shipping a separate int8 quant stream and deletes
the whole on-device quant path.

Per 8x8-block-row-pair strip (128 partitions = (brp, b), free = (c,i,w)):
  1 input DMA (2KB runs) -> DVE regroup to z-grouped layout (3 strided
  copies, 4x DVE mode) -> 24 PE transposes (contiguous 128-chunks) ->
  px PSUM f16 -> copies to SBUF (DVE/Pool 2x) -> 6 single-shot fp16
  matmuls per quarter against one block-diag(8*kron(D,D)[zz].T) weight
  (c0|c1 full 128-col + merged c2 pair via block-diag) -> f32 PSUM ->
  f32->int16 cast copies (Act/DVE) -> 2 output DMAs (3KB runs).

Schedule: quarters pipelined with transposes emitted 2 quarters ahead on
PE; input DMAs prefetch on the SP queue, output DMAs issue from the
scalar queue; engine work balanced DVE/Act/Pool so the ~35us of DMA
traffic (6.3MB in + 6.3MB out per core) is the only roofline.
"""

import numpy as np
import concourse.mybir as mybir
import concourse.tile as tile
from concourse import bacc
from concourse.bass_utils import run_bass_kernel_spmd

F32 = mybir.dt.float32
F16 = mybir.dt.float16
I16 = mybir.dt.int16
P = 8
B, C, H, W = 512, 3, 128, 128
NCORES = 8
BSH = B // NCORES          # 64 batches per core
N = (H // P) * (W // P)    # 256 blocks per plane
CZ = C * P * P             # 192
NBR = H // P               # 16 block rows
NSTRIP = NBR // 2          # 8 strips of 2 block rows

Copy = mybir.ActivationFunctionType.Copy


def _zigzag_flat_idx(n=P):
    order = []
    for s in range(2 * n - 1):
        cells = [(r, s - r) for r in range(max(0, s - n + 1), min(s, n - 1) + 1)]
        if s % 2 == 0:
            cells.reverse()
        order.extend(cells)
    return np.array([r * n + c for r, c in order], dtype=np.int32)


def _build_consts(D: np.ndarray):
    ZZ = _zigzag_flat_idx()
    D64 = D.astype(np.float64)
    KD = np.kron(D64, D64)[ZZ, :]          # (64 zz, 64 pix)
    Mt = (16.0 * KD.T).astype(np.float16)  # (pix, zz), x16 output scale
    Wm = np.zeros((128, 128), dtype=np.float16)
    Wm[0:64, 0:64] = Mt
    Wm[64:128, 64:128] = Mt
    bdid = np.eye(128, dtype=np.float16)
    return Wm, bdid


def _build_nc():
    nc = bacc.Bacc("TRN2", target_bir_lowering=False, debug=False)

    img = nc.dram_tensor("img", [BSH, C, H, W], F16, kind="ExternalInput")
    W_d = nc.dram_tensor("Wm", [128, 128], F16, kind="ExternalInput")
    bdid_d = nc.dram_tensor("bdid", [128, 128], F16, kind="ExternalInput")
    # device layout: [strip, brp, b, (Q, c01|c2 packed)]; host reassembles
    out_d = nc.dram_tensor("out", [NSTRIP, 2, BSH, 3072], I16,
                           kind="ExternalOutput")

    # partition = (brp, b), free = (c, (i, w)) with 2KB contiguous runs
    imgv = img[:].rearrange("b c (bp brp i) w -> bp brp b c (i w)",
                            brp=2, i=P)

    with tile.TileContext(nc) as tc:
        with (
            tc.tile_pool(name="const", bufs=1) as constp,
            tc.tile_pool(name="sload", bufs=4) as sload,
            tc.tile_pool(name="greg", bufs=3) as greg,
            tc.tile_pool(name="x", bufs=6) as xp,
            tc.tile_pool(name="sb", bufs=3) as sbp,
            tc.tile_pool(name="px", bufs=2, space="PSUM") as pxp,
            tc.tile_pool(name="po", bufs=2, space="PSUM") as pop,
        ):
            Wm = constp.tile([128, 128], F16)
            bdid = constp.tile([128, 128], F16)

            S_t = [None] * NSTRIP
            G_t = [None] * NSTRIP
            x_t = [None] * (NSTRIP * 4)
            px_t = [None] * (NSTRIP * 4)
            po_t = [None] * (NSTRIP * 2)
            sb_t = [None] * NSTRIP

            def load_s(bp, split=False):
                S = sload.tile([128, 3072], F16, tag="s", name=f"s{bp}")
                if split:
                    # per-c-plane DMAs on three queues so the regroup
                    # copies can start as early as possible
                    Sv3 = S[:].rearrange("p (c iw) -> p c iw", c=3, iw=1024)
                    nc.sync.dma_start(out=Sv3[:, 0], in_=imgv[bp, :, :, 0])
                    nc.gpsimd.dma_start(out=Sv3[:, 1], in_=imgv[bp, :, :, 1])
                    nc.sync.dma_start(out=Sv3[:, 2], in_=imgv[bp, :, :, 2])
                else:
                    nc.sync.dma_start(out=S[:], in_=imgv[bp])
                S_t[bp] = S

            def regroup(bp, part=None):
                # S free (c,i,q,k,j) -> G (q, z, i, j); z = 2k+c | 4+k
                # part 0/1: c01 k-halves; part 2: c2 (emitted first).
                if part in (0, None):
                    G_t[bp] = greg.tile([128, 3072], F16, tag="g",
                                        name=f"g{bp}")
                G = G_t[bp]
                gv = G[:].rearrange("p (q z i j) -> p z q i j",
                                    q=8, z=6, i=P, j=P)
                sv = S_t[bp][:].rearrange("p (c i q k j) -> p k c q i j",
                                          c=3, i=P, q=8, k=2, j=P)
                parts = range(3) if part is None else [part]
                for pt in parts:
                    if pt < 2:
                        nc.vector.tensor_copy(gv[:, 2 * pt:2 * pt + 2],
                                              sv[:, pt, 0:2])
                    else:
                        nc.vector.tensor_copy(gv[:, 4:6], sv[:, :, 2])

            def transposes(u):
                bp, Q = u // 4, u % 4
                G = G_t[bp]
                px = pxp.tile([128, 768], F16, tag="px", name=f"px{u}")
                for t in range(2):
                    q = Q * 2 + t
                    for k in range(2):
                        nc.tensor.matmul(
                            px[:, (t * 2 + k) * 128:(t * 2 + k + 1) * 128],
                            G[:, q * 384 + k * 128:q * 384 + (k + 1) * 128],
                            bdid[:], is_transpose=True)
                    nc.tensor.matmul(
                        px[:, 512 + t * 128:512 + (t + 1) * 128],
                        G[:, q * 384 + 256:q * 384 + 384],
                        bdid[:], is_transpose=True)
                px_t[u] = px

            def xcopy(u):
                bp, Q = u // 4, u % 4
                x = xp.tile([128, 768], F16, tag="x", name=f"x{u}")
                nc.vector.tensor_copy(x[:], px_t[u][:])
                x_t[u] = x

            def matmuls(u):
                # half-strip po tile: [c01 8x128 | c2 4x128] per half
                x = x_t[u]
                h, lq = u // 2, u % 2
                if lq == 0:
                    po_t[h] = pop.tile([128, 1536], F32, tag="po",
                                       name=f"po{h}")
                po = po_t[h]
                for t in range(4):
                    bwl = lq * 4 + t
                    nc.tensor.matmul(po[:, bwl * 128:(bwl + 1) * 128],
                                     x[:, t * 128:(t + 1) * 128], Wm[:],
                                     start=True, stop=True)
                for t in range(2):
                    p = lq * 2 + t
                    nc.tensor.matmul(
                        po[:, 1024 + p * 128:1024 + (p + 1) * 128],
                        x[:, 512 + t * 128:512 + (t + 1) * 128],
                        Wm[:], start=True, stop=True)

            def outcopy(h, last=False):
                bp, hh = h // 2, h % 2
                if hh == 0:
                    sb_t[bp] = sbp.tile([128, 3072], I16, tag="sb",
                                        name=f"sb{bp}")
                dst = sb_t[bp][:, hh * 1536:(hh + 1) * 1536]
                if last:
                    # final half: split Act/DVE to shorten the tail chain
                    nc.scalar.activation(dst.rearrange(
                        "p (a f) -> p a f", a=2)[:, 0], po_t[h][:, 0:768],
                        Copy)
                    nc.vector.tensor_copy(dst.rearrange(
                        "p (a f) -> p a f", a=2)[:, 1], po_t[h][:, 768:1536])
                else:
                    nc.scalar.activation(dst, po_t[h][:], Copy)

            def out_dma(bp, half, last=False):
                if last:
                    # final half: two parallel-queue DMAs to shorten tail
                    nc.gpsimd.dma_start(out=out_d[bp, :, :, 1536:2304],
                                        in_=sb_t[bp][:, 1536:2304])
                    nc.sync.dma_start(out=out_d[bp, :, :, 2304:3072],
                                      in_=sb_t[bp][:, 2304:3072])
                    return
                fsl = slice(half * 1536, (half + 1) * 1536)
                nc.gpsimd.dma_start(out=out_d[bp, :, :, fsl],
                                    in_=sb_t[bp][:, fsl])

            # ---- software-pipelined emission over 32 quarters ----
            # consts first: bdid gates the very first transpose
            nc.scalar.dma_start(out=bdid[:], in_=bdid_d[:])
            nc.scalar.dma_start(out=Wm[:], in_=W_d[:])
            load_s(0, split=True)
            regroup(0)
            load_s(1)
            transposes(0)
            transposes(1)
            NU = NSTRIP * 4
            for u in range(NU):
                bp, Q = u // 4, u % 4
                if Q == 2 and bp + 2 < NSTRIP:
                    load_s(bp + 2)
                xcopy(u)
                if Q < 3 and bp + 1 < NSTRIP:
                    # Q0 -> c2 on gpsimd (ahead of this strip's out-DMAs in
                    # the gpsimd queue), Q1/Q2 -> c01 halves on DVE
                    regroup(bp + 1, part=(0, 1, 2)[Q])
                matmuls(u)
                if u + 2 < NU:
                    transposes(u + 2)
                if Q in (1, 3):
                    h = u // 2
                    last = u == NU - 1
                    outcopy(h, last=last)
                    out_dma(bp, h % 2, last=last)

    nc.compile()
    return nc


_NC_CACHE = None


def _get_nc():
    global _NC_CACHE
    if _NC_CACHE is None:
        _NC_CACHE = _build_nc()
    return _NC_CACHE


def _build_perm():
    # device free offset for (bw, czz): half h = bw//8, local bwl = bw%8;
    # c01 at h*1536 + bwl*128 + c*64 + zz; c2 at h*1536 + 1024 + bwl*64 + zz
    perm = np.zeros(NBR * CZ, dtype=np.int64)
    for bw in range(16):
        h, bwl = bw // 8, bw % 8
        for c in range(3):
            for zz in range(64):
                col = bw * CZ + c * 64 + zz
                if c < 2:
                    off = h * 1536 + bwl * 128 + c * 64 + zz
                else:
                    off = h * 1536 + 1024 + bwl * 64 + zz
                perm[col] = off
    return perm


_PERM = _build_perm()


def kernel(img, D, Q):
    img = np.asarray(img, dtype=np.float32)
    D = np.asarray(D, dtype=np.float32)
    Q = np.asarray(Q, dtype=np.float32)
    Wm, bdid = _build_consts(D)
    ZZ = _zigzag_flat_idx()
    q_zz = np.tile(Q.flatten()[ZZ], C).astype(np.float32)     # (192,)

    # subtract 128 on host: halves fp16 input/weight noise and keeps
    # |16*dct| <= 16384 in int16 (dct of X-128 matches the reference)
    img16 = np.ascontiguousarray(img - np.float32(128.0)).astype(np.float16)
    nc = _get_nc()
    in_maps = [
        {"img": img16[kk * BSH:(kk + 1) * BSH], "Wm": Wm, "bdid": bdid}
        for kk in range(NCORES)
    ]
    res = run_bass_kernel_spmd(nc, in_maps, core_ids=list(range(NCORES)))

    parts = []
    for r in res.results:
        dev = np.asarray(r["out"])                 # (8, 2, 64, 3072) i16
        f = dev[..., _PERM].astype(np.float32)     # (8, 2, 64, 16*192)
        f = f.reshape(NSTRIP, 2, BSH, NBR, CZ)
        f = f.transpose(0, 1, 3, 2, 4).reshape(N, BSH, CZ)
        parts.append(f)
    nq = np.concatenate(parts, axis=1) * np.float32(0.0625)   # (256, 512, 192)
    flatten = np.round(nq / q_zz)
    return (flatten, nq)


# revision 35
# speedup vs baseline: 1.0075x; 1.0074x over previous
"""JPEG encoder Bass kernel for TRN2 — self-contained, 8-core data-parallel.

kernel(img, D, Q) -> (flatten, no_quan_flatten), matching the reference:
    per 8x8 block: dct = D @ (X - 128) @ D.T ; quant = round(dct / Q);
    both zigzag-gathered + channel-concatenated to (256, 512, 192).

Device computes ONLY the unquantized DCT as int16 (= 8 * dct(X), which
always fits: |8*dct| <= 16320); the host derives nq = int16/8 with a DC
fix (dct(X) = dct(X-128) + 1024*delta_00) and quant = round(nq/Q). This
halves output DMA vs shipping a separate int8 quant stream and deletes
the whole on-device quant path.

Per 8x8-block-row-pair strip (128 partitions = (brp, b), free = (c,i,w)):
  1 input DMA (2KB runs) -> DVE regroup to z-grouped layout (3 strided
  copies, 4x DVE mode) -> 24 PE transposes (contiguous 128-chunks) ->
  px PSUM f16 -> copies to SBUF (DVE/Pool 2x) -> 6 single-shot fp16
  matmuls per quarter against one block-diag(8*kron(D,D)[zz].T) weight
  (c0|c1 full 128-col + merged c2 pair via block-diag) -> f32 PSUM ->
  f32->int16 cast copies (Act/DVE) -> 2 output DMAs (3KB runs).

Schedule: quarters pipelined with transposes emitted 2 quarters ahead on
PE; input DMAs prefetch on the SP queue, output DMAs issue from the
scalar queue; engine work balanced DVE/Act/Pool so the ~35us of DMA
traffic (6.3MB in + 6.3MB out per core) is the only roofline.
"""

import numpy as np
import concourse.mybir as mybir
import concourse.tile as tile
from concourse import bacc
from concourse.bass_utils import run_bass_kernel_spmd

F32 = mybir.dt.float32
F16 = mybir.dt.float16
I16 = mybir.dt.int16
P = 8
B, C, H, W = 512, 3, 128, 128
NCORES = 8
BSH = B // NCORES          # 64 batches per core
N = (H // P) * (W // P)    # 256 blocks per plane
CZ = C * P * P             # 192
NBR = H // P               # 16 block rows
NSTRIP = NBR // 2          # 8 strips of 2 block rows

Copy = mybir.ActivationFunctionType.Copy


def _zigzag_flat_idx(n=P):
    order = []
    for s in range(2 * n - 1):
        cells = [(r, s - r) for r in range(max(0, s - n + 1), min(s, n - 1) + 1)]
        if s % 2 == 0:
            cells.reverse()
        order.extend(cells)
    return np.array([r * n + c for r, c in order], dtype=np.int32)


def _build_consts(D: np.ndarray):
    ZZ = _zigzag_flat_idx()
    D64 = D.astype(np.float64)
    KD = np.kron(D64, D64)[ZZ, :]          # (64 zz, 64 pix)
    Mt = (16.0 * KD.T).astype(np.float16)  # (pix, zz), x16 output scale
    Wm = np.zeros((128, 128), dtype=np.float16)
    Wm[0:64, 0:64] = Mt
    Wm[64:128, 64:128] = Mt
    bdid = np.eye(128, dtype=np.float16)
    return Wm, bdid


def _build_nc():
    nc = bacc.Bacc("TRN2", target_bir_lowering=False, debug=False)

    img = nc.dram_tensor("img", [BSH, C, H, W], F16, kind="ExternalInput")
    W_d = nc.dram_tensor("Wm", [128, 128], F16, kind="ExternalInput")
    bdid_d = nc.dram_tensor("bdid", [128, 128], F16, kind="ExternalInput")
    # device layout: [strip, brp, b, (Q, c01|c2 packed)]; host reassembles
    out_d = nc.dram_tensor("out", [NSTRIP, 2, BSH, 3072], I16,
                           kind="ExternalOutput")

    # partition = (brp, b), free = (c, (i, w)) with 2KB contiguous runs
    imgv = img[:].rearrange("b c (bp brp i) w -> bp brp b c (i w)",
                            brp=2, i=P)

    with tile.TileContext(nc) as tc:
        with (
            tc.tile_pool(name="const", bufs=1) as constp,
            tc.tile_pool(name="sload", bufs=4) as sload,
            tc.tile_pool(name="greg", bufs=3) as greg,
            tc.tile_pool(name="x", bufs=6) as xp,
            tc.tile_pool(name="sb", bufs=3) as sbp,
            tc.tile_pool(name="px", bufs=2, space="PSUM") as pxp,
            tc.tile_pool(name="po", bufs=2, space="PSUM") as pop,
        ):
            Wm = constp.tile([128, 128], F16)
            bdid = constp.tile([128, 128], F16)

            S_t = [None] * NSTRIP
            G_t = [None] * NSTRIP
            x_t = [None] * (NSTRIP * 4)
            px_t = [None] * (NSTRIP * 4)
            po_t = [None] * (NSTRIP * 2)
            sb_t = [None] * NSTRIP

            def load_s(bp, split=False):
                S = sload.tile([128, 3072], F16, tag="s", name=f"s{bp}")
                if split:
                    # per-c-plane DMAs on three queues so the regroup
                    # copies can start as early as possible
                    Sv3 = S[:].rearrange("p (c iw) -> p c iw", c=3, iw=1024)
                    nc.sync.dma_start(out=Sv3[:, 0], in_=imgv[bp, :, :, 0])
                    nc.gpsimd.dma_start(out=Sv3[:, 1], in_=imgv[bp, :, :, 1])
                    nc.sync.dma_start(out=Sv3[:, 2], in_=imgv[bp, :, :, 2])
                else:
                    nc.sync.dma_start(out=S[:], in_=imgv[bp])
                S_t[bp] = S

            def regroup(bp, part=None):
                # S free (c,i,q,k,j) -> G (q, z, i, j); z = 2k+c | 4+k
                # part 0/1: c01 k-halves; part 2: c2 (emitted first).
                if part in (2, None):
                    G_t[bp] = greg.tile([128, 3072], F16, tag="g",
                                        name=f"g{bp}")
                G = G_t[bp]
                gv = G[:].rearrange("p (q z i j) -> p z q i j",
                                    q=8, z=6, i=P, j=P)
                sv = S_t[bp][:].rearrange("p (c i q k j) -> p k c q i j",
                                          c=3, i=P, q=8, k=2, j=P)
                parts = range(3) if part is None else [part]
                for pt in parts:
                    if pt < 2:
                        nc.vector.tensor_copy(gv[:, 2 * pt:2 * pt + 2],
                                              sv[:, pt, 0:2])
                    elif part is None:
                        nc.vector.tensor_copy(gv[:, 4:6], sv[:, :, 2])
                    else:
                        # c2 regroup on gpsimd, emitted ahead of this
                        # strip's out-DMAs in the gpsimd queue
                        nc.gpsimd.tensor_copy(gv[:, 4:6], sv[:, :, 2])

            def transposes(u):
                bp, Q = u // 4, u % 4
                G = G_t[bp]
                px = pxp.tile([128, 768], F16, tag="px", name=f"px{u}")
                for t in range(2):
                    q = Q * 2 + t
                    for k in range(2):
                        nc.tensor.matmul(
                            px[:, (t * 2 + k) * 128:(t * 2 + k + 1) * 128],
                            G[:, q * 384 + k * 128:q * 384 + (k + 1) * 128],
                            bdid[:], is_transpose=True)
                    nc.tensor.matmul(
                        px[:, 512 + t * 128:512 + (t + 1) * 128],
                        G[:, q * 384 + 256:q * 384 + 384],
                        bdid[:], is_transpose=True)
                px_t[u] = px

            def xcopy(u):
                bp, Q = u // 4, u % 4
                x = xp.tile([128, 768], F16, tag="x", name=f"x{u}")
                nc.vector.tensor_copy(x[:], px_t[u][:])
                x_t[u] = x

            def matmuls(u):
                # half-strip po tile: [c01 8x128 | c2 4x128] per half
                x = x_t[u]
                h, lq = u // 2, u % 2
                if lq == 0:
                    po_t[h] = pop.tile([128, 1536], F32, tag="po",
                                       name=f"po{h}")
                po = po_t[h]
                for t in range(4):
                    bwl = lq * 4 + t
                    nc.tensor.matmul(po[:, bwl * 128:(bwl + 1) * 128],
                                     x[:, t * 128:(t + 1) * 128], Wm[:],
                                     start=True, stop=True)
                for t in range(2):
                    p = lq * 2 + t
                    nc.tensor.matmul(
                        po[:, 1024 + p * 128:1024 + (p + 1) * 128],
                        x[:, 512 + t * 128:512 + (t + 1) * 128],
                        Wm[:], start=True, stop=True)

            def outcopy(h, last=False):
                bp, hh = h // 2, h % 2
                if hh == 0:
                    sb_t[bp] = sbp.tile([128, 3072], I16, tag="sb",
                                        name=f"sb{bp}")
                dst = sb_t[bp][:, hh * 1536:(hh + 1) * 1536]
                if last:
                    # final half: split Act/DVE to shorten the tail chain
                    nc.scalar.activation(dst.rearrange(
                        "p (a f) -> p a f", a=2)[:, 0], po_t[h][:, 0:768],
                        Copy)
                    nc.vector.tensor_copy(dst.rearrange(
                        "p (a f) -> p a f", a=2)[:, 1], po_t[h][:, 768:1536])
                else:
                    nc.scalar.activation(dst, po_t[h][:], Copy)

            def out_dma(bp, half, last=False):
                if last:
                    # final half: two parallel-queue DMAs to shorten tail
                    nc.gpsimd.dma_start(out=out_d[bp, :, :, 1536:2304],
                                        in_=sb_t[bp][:, 1536:2304])
                    nc.sync.dma_start(out=out_d[bp, :, :, 2304:3072],
                                      in_=sb_t[bp][:, 2304:3072])
                    return
                fsl = slice(half * 1536, (half + 1) * 1536)
                nc.gpsimd.dma_start(out=out_d[bp, :, :, fsl],
                                    in_=sb_t[bp][:, fsl])

            # ---- software-pipelined emission over 32 quarters ----
            # consts first: bdid gates the very first transpose
            nc.scalar.dma_start(out=bdid[:], in_=bdid_d[:])
            nc.scalar.dma_start(out=Wm[:], in_=W_d[:])
            load_s(0, split=True)
            regroup(0)
            load_s(1)
            transposes(0)
            transposes(1)
            NU = NSTRIP * 4
            for u in range(NU):
                bp, Q = u // 4, u % 4
                if Q == 2 and bp + 2 < NSTRIP:
                    load_s(bp + 2)
                xcopy(u)
                if Q < 3 and bp + 1 < NSTRIP:
                    # Q0 -> c2 on gpsimd (ahead of this strip's out-DMAs in
                    # the gpsimd queue), Q1/Q2 -> c01 halves on DVE
                    regroup(bp + 1, part=(2, 0, 1)[Q])
                matmuls(u)
                if u + 2 < NU:
                    transposes(u + 2)
                if Q in (1, 3):
                    h = u // 2
                    last = u == NU - 1
                    outcopy(h, last=last)
                    out_dma(bp, h % 2, last=last)

    nc.compile()
    return nc


_NC_CACHE = None


def _get_nc():
    global _NC_CACHE
    if _NC_CACHE is None:
        _NC_CACHE = _build_nc()
    return _NC_CACHE


def _build_perm():
    # device free offset for (bw, czz): half h = bw//8, local bwl = bw%8;
    # c01 at h*1536 + bwl*128 + c*64 + zz; c2 at h*1536 + 1024 + bwl*64 + zz
    perm = np.zeros(NBR * CZ, dtype=np.int64)
    for bw in range(16):
        h, bwl = bw // 8, bw % 8
        for c in range(3):
            for zz in range(64):
                col = bw * CZ + c * 64 + zz
                if c < 2:
                    off = h * 1536 + bwl * 128 + c * 64 + zz
                else:
                    off = h * 1536 + 1024 + bwl * 64 + zz
                perm[col] = off
    return perm


_PERM = _build_perm()


def kernel(img, D, Q):
    img = np.asarray(img, dtype=np.float32)
    D = np.asarray(D, dtype=np.float32)
    Q = np.asarray(Q, dtype=np.float32)
    Wm, bdid = _build_consts(D)
    ZZ = _zigzag_flat_idx()
    q_zz = np.tile(Q.flatten()[ZZ], C).astype(np.float32)     # (192,)

    # subtract 128 on host: halves fp16 input/weight noise and keeps
    # |16*dct| <= 16384 in int16 (dct of X-128 matches the reference)
    img16 = np.ascontiguousarray(img - np.float32(128.0)).astype(np.float16)
    nc = _get_nc()
    in_maps = [
        {"img": img16[kk * BSH:(kk + 1) * BSH], "Wm": Wm, "bdid": bdid}
        for kk in range(NCORES)
    ]
    res = run_bass_kernel_spmd(nc, in_maps, core_ids=list(range(NCORES)))

    parts = []
    for r in res.results:
        dev = np.asarray(r["out"])                 # (8, 2, 64, 3072) i16
        f = dev[..., _PERM].astype(np.float32)     # (8, 2, 64, 16*192)
        f = f.reshape(NSTRIP, 2, BSH, NBR, CZ)
        f = f.transpose(0, 1, 3, 2, 4).reshape(N, BSH, CZ)
        parts.append(f)
    nq = np.concatenate(parts, axis=1) * np.float32(0.0625)   # (256, 512, 192)
    flatten = np.round(nq / q_zz)
    return (flatten, nq)


# revision 36
# speedup vs baseline: 1.0136x; 1.0061x over previous
"""JPEG encoder Bass kernel for TRN2 — self-contained, 8-core data-parallel.

kernel(img, D, Q) -> (flatten, no_quan_flatten), matching the reference:
    per 8x8 block: dct = D @ (X - 128) @ D.T ; quant = round(dct / Q);
    both zigzag-gathered + channel-concatenated to (256, 512, 192).

Device computes ONLY the unquantized DCT as int16 (= 8 * dct(X), which
always fits: |8*dct| <= 16320); the host derives nq = int16/8 with a DC
fix (dct(X) = dct(X-128) + 1024*delta_00) and quant = round(nq/Q). This
halves output DMA vs shipping a separate int8 quant stream and deletes
the whole on-device quant path.

Per 8x8-block-row-pair strip (128 partitions = (brp, b), free = (c,i,w)):
  1 input DMA (2KB runs) -> DVE regroup to z-grouped layout (3 strided
  copies, 4x DVE mode) -> 24 PE transposes (contiguous 128-chunks) ->
  px PSUM f16 -> copies to SBUF (DVE/Pool 2x) -> 6 single-shot fp16
  matmuls per quarter against one block-diag(8*kron(D,D)[zz].T) weight
  (c0|c1 full 128-col + merged c2 pair via block-diag) -> f32 PSUM ->
  f32->int16 cast copies (Act/DVE) -> 2 output DMAs (3KB runs).

Schedule: quarters pipelined with transposes emitted 2 quarters ahead on
PE; input DMAs prefetch on the SP queue, output DMAs issue from the
scalar queue; engine work balanced DVE/Act/Pool so the ~35us of DMA
traffic (6.3MB in + 6.3MB out per core) is the only roofline.
"""

import numpy as np
import concourse.mybir as mybir
import concourse.tile as tile
from concourse import bacc
from concourse.bass_utils import run_bass_kernel_spmd

F32 = mybir.dt.float32
F16 = mybir.dt.float16
I16 = mybir.dt.int16
P = 8
B, C, H, W = 512, 3, 128, 128
NCORES = 8
BSH = B // NCORES          # 64 batches per core
N = (H // P) * (W // P)    # 256 blocks per plane
CZ = C * P * P             # 192
NBR = H // P               # 16 block rows
NSTRIP = NBR // 2          # 8 strips of 2 block rows

Copy = mybir.ActivationFunctionType.Copy


def _zigzag_flat_idx(n=P):
    order = []
    for s in range(2 * n - 1):
        cells = [(r, s - r) for r in range(max(0, s - n + 1), min(s, n - 1) + 1)]
        if s % 2 == 0:
            cells.reverse()
        order.extend(cells)
    return np.array([r * n + c for r, c in order], dtype=np.int32)


def _build_consts(D: np.ndarray):
    ZZ = _zigzag_flat_idx()
    D64 = D.astype(np.float64)
    KD = np.kron(D64, D64)[ZZ, :]          # (64 zz, 64 pix)
    Mt = (16.0 * KD.T).astype(np.float16)  # (pix, zz), x16 output scale
    Wm = np.zeros((128, 128), dtype=np.float16)
    Wm[0:64, 0:64] = Mt
    Wm[64:128, 64:128] = Mt
    bdid = np.eye(128, dtype=np.float16)
    return Wm, bdid


def _build_nc():
    nc = bacc.Bacc("TRN2", target_bir_lowering=False, debug=False)

    img = nc.dram_tensor("img", [BSH, C, H, W], F16, kind="ExternalInput")
    W_d = nc.dram_tensor("Wm", [128, 128], F16, kind="ExternalInput")
    bdid_d = nc.dram_tensor("bdid", [128, 128], F16, kind="ExternalInput")
    # device layout: [strip, brp, b, (Q, c01|c2 packed)]; host reassembles
    out_d = nc.dram_tensor("out", [NSTRIP, 2, BSH, 3072], I16,
                           kind="ExternalOutput")

    # partition = (brp, b), free = (c, (i, w)) with 2KB contiguous runs
    imgv = img[:].rearrange("b c (bp brp i) w -> bp brp b c (i w)",
                            brp=2, i=P)

    with tile.TileContext(nc) as tc:
        with (
            tc.tile_pool(name="const", bufs=1) as constp,
            tc.tile_pool(name="sload", bufs=4) as sload,
            tc.tile_pool(name="greg", bufs=3) as greg,
            tc.tile_pool(name="x", bufs=6) as xp,
            tc.tile_pool(name="sb", bufs=3) as sbp,
            tc.tile_pool(name="px", bufs=2, space="PSUM") as pxp,
            tc.tile_pool(name="po", bufs=2, space="PSUM") as pop,
        ):
            Wm = constp.tile([128, 128], F16)
            bdid = constp.tile([128, 128], F16)

            S_t = [None] * NSTRIP
            G_t = [None] * NSTRIP
            x_t = [None] * (NSTRIP * 4)
            px_t = [None] * (NSTRIP * 4)
            po_t = [None] * (NSTRIP * 2)
            sb_t = [None] * NSTRIP

            def load_s(bp, split=False):
                S = sload.tile([128, 3072], F16, tag="s", name=f"s{bp}")
                if split:
                    # per-c-plane DMAs on three queues so the regroup
                    # copies can start as early as possible
                    Sv3 = S[:].rearrange("p (c iw) -> p c iw", c=3, iw=1024)
                    nc.sync.dma_start(out=Sv3[:, 0], in_=imgv[bp, :, :, 0])
                    nc.gpsimd.dma_start(out=Sv3[:, 1], in_=imgv[bp, :, :, 1])
                    nc.sync.dma_start(out=Sv3[:, 2], in_=imgv[bp, :, :, 2])
                else:
                    nc.sync.dma_start(out=S[:], in_=imgv[bp])
                S_t[bp] = S

            def regroup(bp, part=None):
                # S free (c,i,q,k,j) -> G (q, z, i, j); z = 2k+c | 4+k
                # part 0/1: c01 k-halves; part 2: c2 (emitted first).
                if part in (2, None):
                    G_t[bp] = greg.tile([128, 3072], F16, tag="g",
                                        name=f"g{bp}")
                G = G_t[bp]
                gv = G[:].rearrange("p (q z i j) -> p z q i j",
                                    q=8, z=6, i=P, j=P)
                sv = S_t[bp][:].rearrange("p (c i q k j) -> p k c q i j",
                                          c=3, i=P, q=8, k=2, j=P)
                parts = range(3) if part is None else [part]
                for pt in parts:
                    if pt < 2:
                        nc.vector.tensor_copy(gv[:, 2 * pt:2 * pt + 2],
                                              sv[:, pt, 0:2])
                    elif part is None:
                        nc.vector.tensor_copy(gv[:, 4:6], sv[:, :, 2])
                    else:
                        # c2 regroup on gpsimd, emitted ahead of this
                        # strip's out-DMAs in the gpsimd queue
                        nc.gpsimd.tensor_copy(gv[:, 4:6], sv[:, :, 2])

            def transposes(u):
                bp, Q = u // 4, u % 4
                G = G_t[bp]
                px = pxp.tile([128, 768], F16, tag="px", name=f"px{u}")
                for t in range(2):
                    q = Q * 2 + t
                    for k in range(2):
                        nc.tensor.matmul(
                            px[:, (t * 2 + k) * 128:(t * 2 + k + 1) * 128],
                            G[:, q * 384 + k * 128:q * 384 + (k + 1) * 128],
                            bdid[:], is_transpose=True)
                    nc.tensor.matmul(
                        px[:, 512 + t * 128:512 + (t + 1) * 128],
                        G[:, q * 384 + 256:q * 384 + 384],
                        bdid[:], is_transpose=True)
                px_t[u] = px

            def xcopy(u):
                bp, Q = u // 4, u % 4
                x = xp.tile([128, 768], F16, tag="x", name=f"x{u}")
                nc.vector.tensor_copy(x[:], px_t[u][:])
                x_t[u] = x

            def matmuls(u):
                # half-strip po tile: [c01 8x128 | c2 4x128] per half
                x = x_t[u]
                h, lq = u // 2, u % 2
                if lq == 0:
                    po_t[h] = pop.tile([128, 1536], F32, tag="po",
                                       name=f"po{h}")
                po = po_t[h]
                for t in range(4):
                    bwl = lq * 4 + t
                    nc.tensor.matmul(po[:, bwl * 128:(bwl + 1) * 128],
                                     x[:, t * 128:(t + 1) * 128], Wm[:],
                                     start=True, stop=True)
                for t in range(2):
                    p = lq * 2 + t
                    nc.tensor.matmul(
                        po[:, 1024 + p * 128:1024 + (p + 1) * 128],
                        x[:, 512 + t * 128:512 + (t + 1) * 128],
                        Wm[:], start=True, stop=True)

            def outcopy(h, last=False):
                bp, hh = h // 2, h % 2
                if hh == 0:
                    sb_t[bp] = sbp.tile([128, 3072], I16, tag="sb",
                                        name=f"sb{bp}")
                dst = sb_t[bp][:, hh * 1536:(hh + 1) * 1536]
                if last:
                    # last strip: split Act/DVE to shorten the tail chain
                    nc.scalar.activation(dst.rearrange(
                        "p (a f) -> p a f", a=2)[:, 0], po_t[h][:, 0:768],
                        Copy)
                    nc.vector.tensor_copy(dst.rearrange(
                        "p (a f) -> p a f", a=2)[:, 1], po_t[h][:, 768:1536])
                else:
                    nc.scalar.activation(dst, po_t[h][:], Copy)

            def out_dma(bp, half, last=False):
                base = half * 1536
                if last:
                    # last strip: two parallel-queue DMAs to shorten tail
                    q2 = nc.sync if half else nc.scalar
                    nc.gpsimd.dma_start(
                        out=out_d[bp, :, :, base:base + 768],
                        in_=sb_t[bp][:, base:base + 768])
                    q2.dma_start(
                        out=out_d[bp, :, :, base + 768:base + 1536],
                        in_=sb_t[bp][:, base + 768:base + 1536])
                    return
                fsl = slice(base, base + 1536)
                nc.gpsimd.dma_start(out=out_d[bp, :, :, fsl],
                                    in_=sb_t[bp][:, fsl])

            # ---- software-pipelined emission over 32 quarters ----
            # consts first: bdid gates the very first transpose
            nc.scalar.dma_start(out=bdid[:], in_=bdid_d[:])
            nc.scalar.dma_start(out=Wm[:], in_=W_d[:])
            load_s(0, split=True)
            regroup(0)
            load_s(1)
            transposes(0)
            transposes(1)
            NU = NSTRIP * 4
            for u in range(NU):
                bp, Q = u // 4, u % 4
                if Q == 2 and bp + 2 < NSTRIP:
                    load_s(bp + 2)
                xcopy(u)
                if Q < 3 and bp + 1 < NSTRIP:
                    # Q0 -> c2 on gpsimd (ahead of this strip's out-DMAs in
                    # the gpsimd queue), Q1/Q2 -> c01 halves on DVE
                    regroup(bp + 1, part=(2, 0, 1)[Q])
                matmuls(u)
                if u + 2 < NU:
                    transposes(u + 2)
                if Q in (1, 3):
                    h = u // 2
                    last = bp == NSTRIP - 1
                    outcopy(h, last=last)
                    out_dma(bp, h % 2, last=last)

    nc.compile()
    return nc


_NC_CACHE = None


def _get_nc():
    global _NC_CACHE
    if _NC_CACHE is None:
        _NC_CACHE = _build_nc()
    return _NC_CACHE


def _build_perm():
    # device free offset for (bw, czz): half h = bw//8, local bwl = bw%8;
    # c01 at h*1536 + bwl*128 + c*64 + zz; c2 at h*1536 + 1024 + bwl*64 + zz
    perm = np.zeros(NBR * CZ, dtype=np.int64)
    for bw in range(16):
        h, bwl = bw // 8, bw % 8
        for c in range(3):
            for zz in range(64):
                col = bw * CZ + c * 64 + zz
                if c < 2:
                    off = h * 1536 + bwl * 128 + c * 64 + zz
                else:
                    off = h * 1536 + 1024 + bwl * 64 + zz
                perm[col] = off
    return perm


_PERM = _build_perm()


def kernel(img, D, Q):
    img = np.asarray(img, dtype=np.float32)
    D = np.asarray(D, dtype=np.float32)
    Q = np.asarray(Q, dtype=np.float32)
    Wm, bdid = _build_consts(D)
    ZZ = _zigzag_flat_idx()
    q_zz = np.tile(Q.flatten()[ZZ], C).astype(np.float32)     # (192,)

    # subtract 128 on host: halves fp16 input/weight noise and keeps
    # |16*dct| <= 16384 in int16 (dct of X-128 matches the reference)
    img16 = np.ascontiguousarray(img - np.float32(128.0)).astype(np.float16)
    nc = _get_nc()
    in_maps = [
        {"img": img16[kk * BSH:(kk + 1) * BSH], "Wm": Wm, "bdid": bdid}
        for kk in range(NCORES)
    ]
    res = run_bass_kernel_spmd(nc, in_maps, core_ids=list(range(NCORES)))

    parts = []
    for r in res.results:
        dev = np.asarray(r["out"])                 # (8, 2, 64, 3072) i16
        f = dev[..., _PERM].astype(np.float32)     # (8, 2, 64, 16*192)
        f = f.reshape(NSTRIP, 2, BSH, NBR, CZ)
        f = f.transpose(0, 1, 3, 2, 4).reshape(N, BSH, CZ)
        parts.append(f)
    nq = np.concatenate(parts, axis=1) * np.float32(0.0625)   # (256, 512, 192)
    flatten = np.round(nq / q_zz)
    return (flatten, nq)
